# revision 1
# baseline (speedup 1.0000x reference)
"""Trainium2 Bass kernel for loss = sum((X[:,None]*A - I)**2), N=8192.

Algebraic decomposition (avoids materializing the residual):
    loss = sum_ij (x_i*a_ij)^2  -  2*sum_i x_i*a_ii  +  N
         = sum_i x_i^2 * r_i    -  2*sum_i x_i*d_i   +  N
where r_i = sum_j a_ij^2 (row sums of squares) and d_i = a_ii.

The device computes only sum_i x_i^2 * r_i per core; the -2*sum x_i*d_i and
+N terms are folded on the host in float64 (host already has X and diag(A)).

Sharding: A row-wise across 8 cores (1024 rows each). Each core streams its
32 MB shard from HBM once in [128, 2048] (1 MiB) chunks; ScalarE's fused
activation(Square, accum_out) computes per-row partial sums of squares in a
single pass per chunk (~2.3 us/chunk vs ~2.8 us/chunk DMA, so the kernel
stays memory-bound; measured ~372 GB/s sustained). The last row-tile's
chunks taper (2048,2048,2048,1792,256 cols) so the final activation on the
critical path is ~0.5 us instead of ~2 us. The epilogue multiplies the
per-chunk accumulators by host-precomputed x^2 columns, row-reduces to
[128,1], then uses a ones-vector matmul on the (otherwise idle) TensorE to
reduce across partitions to a single [1,1] scalar. That keeps the output
DMA to ONE descriptor: a [128,1] output would fan into 128 4-byte
descriptors whose serialized HBM write receipts cost ~10 us at kernel tail
(measured). The host sums the 8 per-core scalars in float64.
"""

import numpy as np

import concourse.bacc as bacc
import concourse.mybir as mybir
from concourse.tile import TileContext
from concourse.bass_utils import run_bass_kernel_spmd

N = 8192
NCORES = 8
ROWS = N // NCORES  # 1024 rows per core
P = 128  # SBUF partitions
TILES = ROWS // P  # 8 row-tiles of 128 rows per core
CHUNK = 4096  # [128, 4096] f32 = 2 MiB per body DMA

# Per-tile column splits. Body tiles use 2 MiB chunks: ScalarE's
# square+accumulate cost, cost(w) = (w+352)/1.2GHz + ~0.22 us readacc/
# handshake, stays under the chunk DMA time (~4.9 us at the observed
# 420 GB/s single-core peak), so compute never gates the stream. At full
# stream rate the tail chain runs serial once chunk data outpaces ScalarE,
# so the end-of-kernel overhang is
#   max_k [ sum_{j>=k} cost(w_j) - sum_{j>k} dma(w_j) ].
# The last TWO tiles use 1 MiB chunks: with only one 1 MiB-chunk tile the
# critical k sits at the final 2 MiB body activation (overhang ~2.8 us);
# with two, every chain-head term stays under cost(last chunk) ~2.2 us.
# Uniform 2048 is also the RATE-ROBUST optimum: its critical term,
# cost(last), is independent of the stream rate, while any smaller final
# chunk (e.g. ...2176,1920 saving ~0.1 us at 421 GB/s) exposes the
# second-to-last chunk's compute when the end-of-stream rate rises toward
# the ~456 GB/s seen as other cores drain (+0.4 us there).
# Measured dead ends (do not revisit): finer tapers are WORSE (each chunk
# adds ~0.5 us fixed cost to the serial chain); ANY VectorE tail chunk is
# WORSE (DVE square+reduce runs far below line rate and every ScalarE
# ACTIVATE in the kernel inflates ~20% when DVE tail ops exist, v5/v8);
# fused vector.tensor_tensor_reduce crashes the NEFF at execution (v7).
_BODY_SPLIT = [4096, 4096]
_TAIL_SPLIT = [2048, 2048, 2048, 2048]
_SPLITS = [_BODY_SPLIT] * (TILES - 2) + [_TAIL_SPLIT] * 2
NCHUNK = sum(len(s) for s in _SPLITS)  # 20 accumulator columns

_DT = mybir.dt.float32


def build_nc():
    nc = bacc.Bacc("TRN2", target_bir_lowering=False)

    a_shard = nc.dram_tensor("a_shard", [ROWS, N], _DT, kind="ExternalInput")
    # x2c[p, k] = X[shard row t*128+p]**2 for every accumulator column k of
    # row-tile t, so that sum_k (racc*x2c)[p, k] = x^2 * r for that row.
    x2c = nc.dram_tensor("x2c", [P, NCHUNK], _DT, kind="ExternalInput")
    out = nc.dram_tensor("out", [1, 1], _DT, kind="ExternalOutput")

    a_tiles = a_shard.rearrange("(t p) n -> t p n", p=P)

    with TileContext(nc) as tc:
        with (
            tc.tile_pool(name="a", bufs=8) as apool,
            tc.tile_pool(name="small", bufs=1) as small,
            tc.tile_pool(name="ps", bufs=1, space="PSUM") as pspool,
        ):
            racc = small.tile([P, NCHUNK], _DT, tag="racc")
            x2t = small.tile([P, NCHUNK], _DT, tag="x2")
            ones = small.tile([P, 1], _DT, tag="ones")
            nc.gpsimd.memset(ones[:], 1.0)

            # Throwaway full-size output for the fused square+reduce:
            # stride-0 broadcast of a [P,1] tile, so no [P,CHUNK] scratch is
            # needed (qr.py's safe_norm trick).
            dummy = small.tile([P, 1], _DT, tag="dummy")

            k = 0
            for t in range(TILES):
                col = 0
                for w in _SPLITS[t]:
                    at = apool.tile([P, CHUNK], _DT, tag="a")
                    nc.sync.dma_start(
                        out=at[:, :w], in_=a_tiles[t][:, col : col + w]
                    )
                    nc.scalar.activation(
                        out=dummy.broadcast_to((P, w)),
                        in_=at[:, :w],
                        func=mybir.ActivationFunctionType.Square,
                        accum_out=racc[:, k : k + 1],
                    )
                    col += w
                    k += 1
                    if t == 5 and col == 4096:
                        # The x^2 constant is only needed by the epilogue.
                        # Issuing it late (ACT HWDGE ring, mid-stream) keeps
                        # its 128 small descriptors from interleaving with
                        # the A stream near its start or its tail.
                        nc.scalar.dma_start(out=x2t[:], in_=x2c[:])

            # Epilogue: per-partition partials, then cross-partition reduce
            # on TensorE (ones^T @ comb) so the output DMA is 1 descriptor.
            y = small.tile([P, NCHUNK], _DT, tag="y")
            nc.vector.tensor_mul(out=y[:], in0=racc[:], in1=x2t[:])
            comb = small.tile([P, 1], _DT, tag="comb")
            nc.vector.reduce_sum(comb[:], y[:], axis=mybir.AxisListType.X)
            ps = pspool.tile([1, 1], _DT, tag="ps")
            nc.tensor.matmul(ps[:], ones[:], comb[:], start=True, stop=True)
            res = small.tile([1, 1], _DT, tag="res")
            nc.vector.tensor_copy(res[:], ps[:])
            nc.sync.dma_start(out=out[:], in_=res[:])

    nc.compile()
    return nc


_nc_cache = {}


def _get_nc():
    if "nc" not in _nc_cache:
        _nc_cache["nc"] = build_nc()
    return _nc_cache["nc"]


def _shard_inputs(X, A):
    X = np.ascontiguousarray(np.asarray(X, dtype=np.float32))
    A = np.ascontiguousarray(np.asarray(A, dtype=np.float32))
    reps = [len(s) for s in _SPLITS]  # accumulator columns per row-tile
    in_maps = []
    for core in range(NCORES):
        r0 = core * ROWS
        xs = X[r0 : r0 + ROWS].reshape(TILES, P).T  # [P, TILES]
        x2 = np.repeat(xs * xs, reps, axis=1)  # [P, NCHUNK]
        in_maps.append(
            {
                "a_shard": A[r0 : r0 + ROWS],
                "x2c": np.ascontiguousarray(x2.astype(np.float32)),
            }
        )
    return in_maps


def _run(inputs, trace=False, all_cores=False):
    nc = _get_nc()
    X = np.asarray(inputs["X"], dtype=np.float64)
    d = np.asarray(inputs["A"]).diagonal().astype(np.float64)
    in_maps = _shard_inputs(inputs["X"], inputs["A"])
    kwargs = {"trace_cores": list(range(NCORES))} if all_cores else {}
    res = run_bass_kernel_spmd(
        nc, in_maps, core_ids=list(range(NCORES)), trace=trace, **kwargs
    )
    partials = np.array(
        [float(r["out"][0, 0]) for r in res.results], dtype=np.float64
    )
    total = np.float32(partials.sum() - 2.0 * float(X @ d) + float(N))
    return np.array(total, dtype=np.float32), res


def kernel(**inputs):
    out, _ = _run(inputs, trace=False)
    return out



# revision 2
# speedup vs baseline: 1.0436x; 1.0436x over previous
"""Trainium2 Bass kernel for loss = sum((X[:,None]*A - I)**2), N=8192.

Algebraic decomposition (avoids materializing the residual):
    loss = sum_ij (x_i*a_ij)^2  -  2*sum_i x_i*a_ii  +  N
         = sum_i x_i^2 * r_i    -  2*sum_i x_i*d_i   +  N
where r_i = sum_j a_ij^2 (row sums of squares) and d_i = a_ii.

The device computes only sum_i x_i^2 * r_i per core; the -2*sum x_i*d_i and
+N terms are folded on the host in float64 (host already has X and diag(A)).

Sharding: A row-wise across 8 cores (1024 rows each). Each core streams its
32 MB shard from HBM once in [128, 2048] (1 MiB) chunks; ScalarE's fused
activation(Square, accum_out) computes per-row partial sums of squares in a
single pass per chunk (~2.3 us/chunk vs ~2.8 us/chunk DMA, so the kernel
stays memory-bound; measured ~372 GB/s sustained). The last row-tile's
chunks taper (2048,2048,2048,1792,256 cols) so the final activation on the
critical path is ~0.5 us instead of ~2 us. The epilogue multiplies the
per-chunk accumulators by host-precomputed x^2 columns, row-reduces to
[128,1], then uses a ones-vector matmul on the (otherwise idle) TensorE to
reduce across partitions to a single [1,1] scalar. That keeps the output
DMA to ONE descriptor: a [128,1] output would fan into 128 4-byte
descriptors whose serialized HBM write receipts cost ~10 us at kernel tail
(measured). The host sums the 8 per-core scalars in float64.
"""

import numpy as np

import concourse.bacc as bacc
import concourse.mybir as mybir
from concourse.tile import TileContext
from concourse.bass_utils import run_bass_kernel_spmd

N = 8192
NCORES = 8
ROWS = N // NCORES  # 1024 rows per core
P = 128  # SBUF partitions
TILES = ROWS // P  # 8 row-tiles of 128 rows per core
CHUNK = 4096  # [128, 4096] f32 = 2 MiB per body DMA

# Per-tile column splits. Body tiles use 2 MiB chunks: ScalarE's
# square+accumulate cost, cost(w) = (w+352)/1.2GHz + ~0.22 us readacc/
# handshake, stays under the chunk DMA time (~4.9 us at the observed
# 420 GB/s single-core peak), so compute never gates the stream. At full
# stream rate the tail chain runs serial once chunk data outpaces ScalarE,
# so the end-of-kernel overhang is
#   max_k [ sum_{j>=k} cost(w_j) - sum_{j>k} dma(w_j) ].
# The last TWO tiles use 1 MiB chunks: with only one 1 MiB-chunk tile the
# critical k sits at the final 2 MiB body activation (overhang ~2.8 us);
# with two, every chain-head term stays under cost(last chunk) ~2.2 us.
# Uniform 2048 is also the RATE-ROBUST optimum: its critical term,
# cost(last), is independent of the stream rate, while any smaller final
# chunk (e.g. ...2176,1920 saving ~0.1 us at 421 GB/s) exposes the
# second-to-last chunk's compute when the end-of-stream rate rises toward
# the ~456 GB/s seen as other cores drain (+0.4 us there).
# Measured dead ends (do not revisit): finer tapers are WORSE (each chunk
# adds ~0.5 us fixed cost to the serial chain); ANY VectorE tail chunk is
# WORSE (DVE square+reduce runs far below line rate and every ScalarE
# ACTIVATE in the kernel inflates ~20% when DVE tail ops exist, v5/v8);
# fused vector.tensor_tensor_reduce crashes the NEFF at execution (v7).
#
# Session 2026-08-11 findings (probe-verified; all "fixes" measured WORSE):
# The 113-117us (vs ~95.5us healthy-core) ceiling is an EXTERNAL defect:
# one SDMA engine (ring slot 15; persistent on core 6 this boot,
# intermittently core 0 and others) runs at ~0.78x due to periodic ~11us
# interference bursts that double its per-descriptor time. The reported HW
# time is the max over traced cores (or core 0 if only it is traced), so
# whichever core hosts the defect sets the score. Probes established:
#  - HWDGE assigns a DMA's descriptors to engines by INDEX (round-robin
#    from engine 0), with engine count = largest divisor of the partition
#    count that is <= 16: [128,w]->16x8, [120,w]->15x8, [112,w]->16x7,
#    [15,w]->15x1, [8,w]->8x1, [127,w]->1x127(!). The defect follows the
#    ENGINE, not the SBUF partition (a [1:128] DMA pushed partitions
#    ==15 mod 16 through engine 0 at full speed on the sick core).
#  - ONLY full [128, w] DMAs stream at line rate (607ns/16KB desc/engine
#    = 27 GB/s x16). [0:120] runs 2.06x slow per desc, [0:112] 1.6x,
#    SWDGE (nc.gpsimd) ~1.5-2x, and sub-16-desc pieces ~3-6x (descs
#    pipeline only within one dma_start's per-engine batch; v3's
#    8x[15]+[8] pieces halved the WHOLE stream: 179us).
#  - Starving engine 15 with [0:120,w] chunks + relocated remainder rows
#    (v4) cost +5.1us/chunk on every healthy engine vs 6.1us/chunk saved
#    on the sick one: 129us. NET LOSS; same for any slow-shape mix.
#  - Cross-core rebalancing (fewer rows on the sick core) is impossible:
#    SPMD shares one NEFF, so per-core byte counts cannot differ.
# Conclusion: this kernel is at the achievable roofline; the gap to
# ~95.5us is the defect landing on the measured core, not kernel slack.
_BODY_SPLIT = [4096, 4096]
_TAIL_SPLIT = [2048, 2048, 2048, 2048]
_SPLITS = [_BODY_SPLIT] * (TILES - 2) + [_TAIL_SPLIT] * 2
NCHUNK = sum(len(s) for s in _SPLITS)  # 20 accumulator columns

_DT = mybir.dt.float32


def build_nc():
    nc = bacc.Bacc("TRN2", target_bir_lowering=False)

    a_shard = nc.dram_tensor("a_shard", [ROWS, N], _DT, kind="ExternalInput")
    # x2c[p, k] = X[shard row t*128+p]**2 for every accumulator column k of
    # row-tile t, so that sum_k (racc*x2c)[p, k] = x^2 * r for that row.
    x2c = nc.dram_tensor("x2c", [P, NCHUNK], _DT, kind="ExternalInput")
    out = nc.dram_tensor("out", [1, 1], _DT, kind="ExternalOutput")

    a_tiles = a_shard.rearrange("(t p) n -> t p n", p=P)

    with TileContext(nc) as tc:
        with (
            tc.tile_pool(name="a", bufs=8) as apool,
            tc.tile_pool(name="small", bufs=1) as small,
            tc.tile_pool(name="ps", bufs=1, space="PSUM") as pspool,
        ):
            racc = small.tile([P, NCHUNK], _DT, tag="racc")
            x2t = small.tile([P, NCHUNK], _DT, tag="x2")
            ones = small.tile([P, 1], _DT, tag="ones")
            nc.gpsimd.memset(ones[:], 1.0)

            # Throwaway full-size output for the fused square+reduce:
            # stride-0 broadcast of a [P,1] tile, so no [P,CHUNK] scratch is
            # needed (qr.py's safe_norm trick).
            dummy = small.tile([P, 1], _DT, tag="dummy")

            k = 0
            for t in range(TILES):
                col = 0
                for w in _SPLITS[t]:
                    at = apool.tile([P, CHUNK], _DT, tag="a")
                    nc.sync.dma_start(
                        out=at[:, :w], in_=a_tiles[t][:, col : col + w]
                    )
                    nc.scalar.activation(
                        out=dummy.broadcast_to((P, w)),
                        in_=at[:, :w],
                        func=mybir.ActivationFunctionType.Square,
                        accum_out=racc[:, k : k + 1],
                    )
                    col += w
                    k += 1
                    if t == 5 and col == 4096:
                        # The x^2 constant is only needed by the epilogue.
                        # Issuing it late (ACT HWDGE ring, mid-stream) keeps
                        # its 128 small descriptors from interleaving with
                        # the A stream near its start or its tail.
                        nc.scalar.dma_start(out=x2t[:], in_=x2c[:])

            # Epilogue: per-partition partials, then cross-partition reduce
            # on TensorE (ones^T @ comb) so the output DMA is 1 descriptor.
            y = small.tile([P, NCHUNK], _DT, tag="y")
            nc.vector.tensor_mul(out=y[:], in0=racc[:], in1=x2t[:])
            comb = small.tile([P, 1], _DT, tag="comb")
            nc.vector.reduce_sum(comb[:], y[:], axis=mybir.AxisListType.X)
            ps = pspool.tile([1, 1], _DT, tag="ps")
            nc.tensor.matmul(ps[:], ones[:], comb[:], start=True, stop=True)
            res = small.tile([1, 1], _DT, tag="res")
            nc.vector.tensor_copy(res[:], ps[:])
            nc.sync.dma_start(out=out[:], in_=res[:])

    nc.compile()
    return nc


_nc_cache = {}


def _get_nc():
    if "nc" not in _nc_cache:
        _nc_cache["nc"] = build_nc()
    return _nc_cache["nc"]


def _shard_inputs(X, A):
    X = np.ascontiguousarray(np.asarray(X, dtype=np.float32))
    A = np.ascontiguousarray(np.asarray(A, dtype=np.float32))
    reps = [len(s) for s in _SPLITS]  # accumulator columns per row-tile
    in_maps = []
    for core in range(NCORES):
        r0 = core * ROWS
        xs = X[r0 : r0 + ROWS].reshape(TILES, P).T  # [P, TILES]
        x2 = np.repeat(xs * xs, reps, axis=1)  # [P, NCHUNK]
        in_maps.append(
            {
                "a_shard": A[r0 : r0 + ROWS],
                "x2c": np.ascontiguousarray(x2.astype(np.float32)),
            }
        )
    return in_maps


def _run(inputs, trace=False, all_cores=False):
    nc = _get_nc()
    X = np.asarray(inputs["X"], dtype=np.float64)
    d = np.asarray(inputs["A"]).diagonal().astype(np.float64)
    in_maps = _shard_inputs(inputs["X"], inputs["A"])
    kwargs = {"trace_cores": list(range(NCORES))} if all_cores else {}
    res = run_bass_kernel_spmd(
        nc, in_maps, core_ids=list(range(NCORES)), trace=trace, **kwargs
    )
    partials = np.array(
        [float(r["out"][0, 0]) for r in res.results], dtype=np.float64
    )
    total = np.float32(partials.sum() - 2.0 * float(X @ d) + float(N))
    return np.array(total, dtype=np.float32), res


def kernel(**inputs):
    out, _ = _run(inputs, trace=False)
    return out



# revision 3
# speedup vs baseline: 1.4506x; 1.3900x over previous
"""Trainium2 Bass kernel for loss = sum((X[:,None]*A - I)**2), N=8192.

Algebraic decomposition (avoids materializing the residual):
    loss = sum_ij (x_i*a_ij)^2  -  2*sum_i x_i*a_ii  +  N
         = sum_i x_i^2 * r_i    -  2*sum_i x_i*d_i   +  N
where r_i = sum_j a_ij^2 (row sums of squares) and d_i = a_ii.

The device computes only sum_i x_i^2 * r_i per core; the -2*sum x_i*d_i and
+N terms are folded on the host in float64 (host already has X and diag(A)).

Sharding: A row-wise across 8 cores (1024 rows each). Each core streams its
32 MB shard from HBM once in [128, 2048] (1 MiB) chunks; ScalarE's fused
activation(Square, accum_out) computes per-row partial sums of squares in a
single pass per chunk (~2.3 us/chunk vs ~2.8 us/chunk DMA, so the kernel
stays memory-bound; measured ~372 GB/s sustained). The last row-tile's
chunks taper (2048,2048,2048,1792,256 cols) so the final activation on the
critical path is ~0.5 us instead of ~2 us. The epilogue multiplies the
per-chunk accumulators by host-precomputed x^2 columns, row-reduces to
[128,1], then uses a ones-vector matmul on the (otherwise idle) TensorE to
reduce across partitions to a single [1,1] scalar. That keeps the output
DMA to ONE descriptor: a [128,1] output would fan into 128 4-byte
descriptors whose serialized HBM write receipts cost ~10 us at kernel tail
(measured). The host sums the 8 per-core scalars in float64.
"""

import numpy as np

import concourse.bacc as bacc
import concourse.mybir as mybir
from concourse.tile import TileContext
from concourse.bass_utils import run_bass_kernel_spmd

N = 8192
NCORES = 8
ROWS = N // NCORES  # 1024 rows per core
P = 128  # SBUF partitions
TILES = ROWS // P  # 8 row-tiles of 128 rows per core
CHUNK = 4096  # [128, 4096] f32 = 2 MiB per body DMA

# Per-tile column splits. Body tiles use 2 MiB chunks: ScalarE's
# square+accumulate cost, cost(w) = (w+352)/1.2GHz + ~0.22 us readacc/
# handshake, stays under the chunk DMA time (~4.9 us at the observed
# 420 GB/s single-core peak), so compute never gates the stream. At full
# stream rate the tail chain runs serial once chunk data outpaces ScalarE,
# so the end-of-kernel overhang is
#   max_k [ sum_{j>=k} cost(w_j) - sum_{j>k} dma(w_j) ].
# The last TWO tiles use 1 MiB chunks: with only one 1 MiB-chunk tile the
# critical k sits at the final 2 MiB body activation (overhang ~2.8 us);
# with two, every chain-head term stays under cost(last chunk) ~2.2 us.
# Uniform 2048 is also the RATE-ROBUST optimum: its critical term,
# cost(last), is independent of the stream rate, while any smaller final
# chunk (e.g. ...2176,1920 saving ~0.1 us at 421 GB/s) exposes the
# second-to-last chunk's compute when the end-of-stream rate rises toward
# the ~456 GB/s seen as other cores drain (+0.4 us there).
# Measured dead ends (do not revisit): finer tapers are WORSE (each chunk
# adds ~0.5 us fixed cost to the serial chain); ANY VectorE tail chunk is
# WORSE (DVE square+reduce runs far below line rate and every ScalarE
# ACTIVATE in the kernel inflates ~20% when DVE tail ops exist, v5/v8);
# fused vector.tensor_tensor_reduce crashes the NEFF at execution (v7).
#
# Session 2026-08-11 findings (probe-verified; all "fixes" measured WORSE):
# The 113-117us (vs ~95.5us healthy-core) ceiling is an EXTERNAL defect:
# one SDMA engine (ring slot 15; persistent on core 6 this boot,
# intermittently core 0 and others) runs at ~0.78x due to periodic ~11us
# interference bursts that double its per-descriptor time. The reported HW
# time is the max over traced cores (or core 0 if only it is traced), so
# whichever core hosts the defect sets the score. Probes established:
#  - HWDGE assigns a DMA's descriptors to engines by INDEX (round-robin
#    from engine 0), with engine count = largest divisor of the partition
#    count that is <= 16: [128,w]->16x8, [120,w]->15x8, [112,w]->16x7,
#    [15,w]->15x1, [8,w]->8x1, [127,w]->1x127(!). The defect follows the
#    ENGINE, not the SBUF partition (a [1:128] DMA pushed partitions
#    ==15 mod 16 through engine 0 at full speed on the sick core).
#  - ONLY full [128, w] DMAs stream at line rate (607ns/16KB desc/engine
#    = 27 GB/s x16). [0:120] runs 2.06x slow per desc, [0:112] 1.6x,
#    SWDGE (nc.gpsimd) ~1.5-2x, and sub-16-desc pieces ~3-6x (descs
#    pipeline only within one dma_start's per-engine batch; v3's
#    8x[15]+[8] pieces halved the WHOLE stream: 179us).
#  - Starving engine 15 with [0:120,w] chunks + relocated remainder rows
#    (v4) cost +5.1us/chunk on every healthy engine vs 6.1us/chunk saved
#    on the sick one: 129us. NET LOSS; same for any slow-shape mix.
#  - Cross-core rebalancing (fewer rows on the sick core) is impossible:
#    SPMD shares one NEFF, so per-core byte counts cannot differ.
# Conclusion: this kernel is at the achievable roofline; the gap to
# ~95.5us is the defect landing on the measured core, not kernel slack.
# Further measured dead ends (second pass, same session):
#  - Replacing the gpsimd memset(ones) with an extra x2c column (kernel
#    GpSimd-free, one engine less in barriers/ceremony): NEUTRAL
#    (95.4-95.8us clean-core, same as baseline).
#  - Moving the final out DMA to the scalar/ACT HWDGE ring (to overlap
#    the exit barrier with the output receipt): -6us REGRESSION on every
#    core (101.6-102.1us) with engine busy unchanged -- a second DMA on
#    the ACT ring perturbs the schedule globally. Keep the ACT ring to
#    exactly one mid-stream DMA (x2c) and the out DMA on nc.sync.
#  - Clean-core span anatomy (fixed costs, not kernel slack): ~6.6us NEFF
#    preamble (sem-init DMAs, TENSOR_LOADs, entry barrier) + ~1.5us
#    DGE+first-byte + ~79.4us stream (431 of 435 GB/s fabric ceiling) +
#    ~7.6us tail (last act 2.5, epilogue 1.0, out-DMA gen+receipt 1.8,
#    TileContext exit ceremony 2.1). The interference lands 108-118us on
#    whichever core it visits; 5 of 7 full runs this session had it
#    somewhere, incl. (core 4, e0) and (core 2, e0) -- it moves across
#    cores AND engine slots, so no static derate can dodge it.
_BODY_SPLIT = [4096, 4096]
_TAIL_SPLIT = [2048, 2048, 2048, 2048]
_SPLITS = [_BODY_SPLIT] * (TILES - 2) + [_TAIL_SPLIT] * 2
NCHUNK = sum(len(s) for s in _SPLITS)  # 20 accumulator columns

_DT = mybir.dt.float32


def build_nc():
    nc = bacc.Bacc("TRN2", target_bir_lowering=False)

    a_shard = nc.dram_tensor("a_shard", [ROWS, N], _DT, kind="ExternalInput")
    # x2c[p, k] = X[shard row t*128+p]**2 for every accumulator column k of
    # row-tile t, so that sum_k (racc*x2c)[p, k] = x^2 * r for that row.
    x2c = nc.dram_tensor("x2c", [P, NCHUNK], _DT, kind="ExternalInput")
    out = nc.dram_tensor("out", [1, 1], _DT, kind="ExternalOutput")

    a_tiles = a_shard.rearrange("(t p) n -> t p n", p=P)

    with TileContext(nc) as tc:
        with (
            tc.tile_pool(name="a", bufs=8) as apool,
            tc.tile_pool(name="small", bufs=1) as small,
            tc.tile_pool(name="ps", bufs=1, space="PSUM") as pspool,
        ):
            racc = small.tile([P, NCHUNK], _DT, tag="racc")
            x2t = small.tile([P, NCHUNK], _DT, tag="x2")
            ones = small.tile([P, 1], _DT, tag="ones")
            nc.gpsimd.memset(ones[:], 1.0)

            # Throwaway full-size output for the fused square+reduce:
            # stride-0 broadcast of a [P,1] tile, so no [P,CHUNK] scratch is
            # needed (qr.py's safe_norm trick).
            dummy = small.tile([P, 1], _DT, tag="dummy")

            k = 0
            for t in range(TILES):
                col = 0
                for w in _SPLITS[t]:
                    at = apool.tile([P, CHUNK], _DT, tag="a")
                    nc.sync.dma_start(
                        out=at[:, :w], in_=a_tiles[t][:, col : col + w]
                    )
                    nc.scalar.activation(
                        out=dummy.broadcast_to((P, w)),
                        in_=at[:, :w],
                        func=mybir.ActivationFunctionType.Square,
                        accum_out=racc[:, k : k + 1],
                    )
                    col += w
                    k += 1
                    if t == 5 and col == 4096:
                        # The x^2 constant is only needed by the epilogue.
                        # Issuing it late (ACT HWDGE ring, mid-stream) keeps
                        # its 128 small descriptors from interleaving with
                        # the A stream near its start or its tail.
                        nc.scalar.dma_start(out=x2t[:], in_=x2c[:])

            # Epilogue: per-partition partials, then cross-partition reduce
            # on TensorE (ones^T @ comb) so the output DMA is 1 descriptor.
            y = small.tile([P, NCHUNK], _DT, tag="y")
            nc.vector.tensor_mul(out=y[:], in0=racc[:], in1=x2t[:])
            comb = small.tile([P, 1], _DT, tag="comb")
            nc.vector.reduce_sum(comb[:], y[:], axis=mybir.AxisListType.X)
            ps = pspool.tile([1, 1], _DT, tag="ps")
            nc.tensor.matmul(ps[:], ones[:], comb[:], start=True, stop=True)
            res = small.tile([1, 1], _DT, tag="res")
            nc.vector.tensor_copy(res[:], ps[:])
            nc.sync.dma_start(out=out[:], in_=res[:])

    nc.compile()
    return nc


_nc_cache = {}


def _get_nc():
    if "nc" not in _nc_cache:
        _nc_cache["nc"] = build_nc()
    return _nc_cache["nc"]


def _shard_inputs(X, A):
    X = np.ascontiguousarray(np.asarray(X, dtype=np.float32))
    A = np.ascontiguousarray(np.asarray(A, dtype=np.float32))
    reps = [len(s) for s in _SPLITS]  # accumulator columns per row-tile
    in_maps = []
    for core in range(NCORES):
        r0 = core * ROWS
        xs = X[r0 : r0 + ROWS].reshape(TILES, P).T  # [P, TILES]
        x2 = np.repeat(xs * xs, reps, axis=1)  # [P, NCHUNK]
        in_maps.append(
            {
                "a_shard": A[r0 : r0 + ROWS],
                "x2c": np.ascontiguousarray(x2.astype(np.float32)),
            }
        )
    return in_maps


def _run(inputs, trace=False, all_cores=False):
    nc = _get_nc()
    X = np.asarray(inputs["X"], dtype=np.float64)
    d = np.asarray(inputs["A"]).diagonal().astype(np.float64)
    in_maps = _shard_inputs(inputs["X"], inputs["A"])
    kwargs = {"trace_cores": list(range(NCORES))} if all_cores else {}
    res = run_bass_kernel_spmd(
        nc, in_maps, core_ids=list(range(NCORES)), trace=trace, **kwargs
    )
    partials = np.array(
        [float(r["out"][0, 0]) for r in res.results], dtype=np.float64
    )
    total = np.float32(partials.sum() - 2.0 * float(X @ d) + float(N))
    return np.array(total, dtype=np.float32), res


def kernel(**inputs):
    out, _ = _run(inputs, trace=False)
    return out

